# revision 15
# baseline (speedup 1.0000x reference)
"""Multi-head attention (dense_transformer) Trainium2 Bass kernel.

Problem: nn_MultiHeadAttention_77137612636791
  B=4, S=2048, d_model=512, H=8 heads, d_k=d_v=64, fp32, ~10% masked.
  reference returns (out [B,S,512], attn [B,H,S,S]).

Sharding: 8 cores = (batch b in 0..3) x (query-half in 0..1).  Each core:
  - projects q (its half) / k / v for all 8 heads of its batch,
  - computes scores TRANSPOSED (keys on partitions, queries on free dim),
  - exp via ScalarE (scale=1/sqrt(dk)), multiplicative {0,1} mask via
    VectorE/GpSimd, denominators via a ones-column appended to V inside the
    attn@V matmul (PE computes the cross-partition sums for free),
  - writes attn probabilities transposed ([H, S_k, S_q/2] per core; host
    reassembles with a transpose),
  - output projection + residual + LayerNorm for its query rows.

All matmuls run as float32r (full-rate fp32 on the PE array).
"""

import os
import sys

import numpy as np

_TRN_REPO = "/opt/trn_rl_repo"
for _p in (os.path.join(_TRN_REPO, "concourse"), _TRN_REPO):
    if _p not in sys.path:
        sys.path.insert(0, _p)

B, S, DM, H, DK = 4, 2048, 512, 8, 64
QH = S // 2          # queries per core
NKC = S // 128       # 16 key chunks of 128
NQT = QH // 128      # 8 query tiles of 128
EPS = 1e-5
SCALE = 1.0 / 8.0    # 1/sqrt(DK)

_prog_cache = {}


def build_program():
    """Build the SPMD Bass program (identical for every core)."""
    from contextlib import ExitStack

    import concourse.bass as bass
    import concourse.mybir as mybir
    import concourse.tile as tile
    from concourse import bacc

    F32 = mybir.dt.float32
    F32R = mybir.dt.float32r
    U8 = mybir.dt.uint8
    EXPF = mybir.ActivationFunctionType.Exp
    SQRTF = mybir.ActivationFunctionType.Sqrt
    SQUAREF = mybir.ActivationFunctionType.Square
    MULT = mybir.AluOpType.mult
    ADD = mybir.AluOpType.add

    nc = bacc.Bacc("TRN2", target_bir_lowering=False, debug=False)

    q_d = nc.dram_tensor("q", [QH, DM], F32R, kind="ExternalInput")
    k_d = nc.dram_tensor("k", [S, DM], F32R, kind="ExternalInput")
    v_d = nc.dram_tensor("v", [S, DM], F32R, kind="ExternalInput")
    mt_d = nc.dram_tensor("mt", [S, QH], U8, kind="ExternalInput")
    wq_d = nc.dram_tensor("wq", [DM, DM], F32R, kind="ExternalInput")
    wk_d = nc.dram_tensor("wk", [DM, DM], F32R, kind="ExternalInput")
    wv_d = nc.dram_tensor("wv", [DM, DM], F32R, kind="ExternalInput")
    wo_d = nc.dram_tensor("wo", [DM, DM], F32R, kind="ExternalInput")
    gb_d = nc.dram_tensor("gb", [128, DM], F32, kind="ExternalInput")
    id_d = nc.dram_tensor("ident", [128, 128], F32R, kind="ExternalInput")
    bb_d = nc.dram_tensor("bb", [128, DM], F32, kind="ExternalInput")
    attn_d = nc.dram_tensor("attn_part", [H, S, QH], F32, kind="ExternalOutput")
    out_d = nc.dram_tensor("out_part", [QH, DM], F32, kind="ExternalOutput")

    def mm(out, lhsT, rhs, start, stop):
        nc.tensor.matmul(out, lhsT, rhs, start=start, stop=stop)

    with tile.TileContext(nc) as tc, ExitStack() as ctx:
        persist = ctx.enter_context(tc.tile_pool(name="persist", bufs=1))

        ident = persist.tile([128, 128], F32R)
        nc.scalar.dma_start(ident, id_d[:, :])
        ones_row = persist.tile([1, 128], F32)
        nc.vector.memset(ones_row, 1.0)
        gb_sb = persist.tile([128, DM], F32)
        nc.scalar.dma_start(gb_sb, gb_d[:, :])
        bb_sb = persist.tile([128, DM], F32)
        nc.scalar.dma_start(bb_sb, bb_d[:, :])
        wo_sb = persist.tile([128, 4, DM], F32R)
        nc.scalar.dma_start(wo_sb, wo_d.ap().rearrange("(c p) n -> p c n", p=128))

        # persistent projected tensors (heads stored in pairs of 64 partitions)
        khT = [persist.tile([128, S], F32R, tag=f"khT{p}", name=f"khT{p}")
               for p in range(4)]
        qhT = [persist.tile([128, QH], F32R, tag=f"qhT{p}", name=f"qhT{p}")
               for p in range(4)]
        vh = [persist.tile([128, H, DK + 1], F32R, tag=f"vh{c}", name=f"vh{c}")
              for c in range(NKC)]
        headsT = [persist.tile([128, QH], F32R, tag=f"hT{p}", name=f"hT{p}")
                  for p in range(4)]
        mt_sb = persist.tile([128, NKC, QH], U8)
        nc.scalar.dma_start(mt_sb, mt_d.ap().rearrange("(c p) m -> p c m", p=128))

        # ---------------- phase 1+2: transpose raw activations, project ----
        with tc.tile_pool(name="stage", bufs=1) as stage, \
             tc.tile_pool(name="ldpool", bufs=4) as ldpool, \
             tc.tile_pool(name="tpp", bufs=4, space="PSUM") as tpp, \
             tc.tile_pool(name="pj", bufs=2, space="PSUM") as pj:

            wq_sb = stage.tile([128, 4, DM], F32R, tag="wq")
            nc.scalar.dma_start(wq_sb, wq_d.ap().rearrange("(c p) n -> p c n", p=128))
            wk_sb = stage.tile([128, 4, DM], F32R, tag="wk")
            nc.scalar.dma_start(wk_sb, wk_d.ap().rearrange("(c p) n -> p c n", p=128))
            wv_sb = stage.tile([128, 4, DM], F32R, tag="wv")
            nc.scalar.dma_start(wv_sb, wv_d.ap().rearrange("(c p) n -> p c n", p=128))

            def load_transposed(src_d, n_rows, tag):
                """[n_rows, 512] DRAM -> SBUF [128, 4, n_rows] transposed."""
                tT = stage.tile([128, 4, n_rows], F32R, tag=tag)
                for st in range(n_rows // 128):
                    nat = ldpool.tile([128, DM], F32R, tag="nat")
                    nc.scalar.dma_start(nat, src_d[st * 128:(st + 1) * 128, :])
                    for c in range(4):
                        pt = tpp.tile([128, 128], F32, tag="tp")
                        mm(pt, nat[:, c * 128:(c + 1) * 128], ident,
                           start=True, stop=True)
                        nc.scalar.copy(tT[:, c, st * 128:(st + 1) * 128], pt)
                return tT

            # K -> khT pairs
            kT = load_transposed(k_d, S, tag="rawA")
            for p in range(4):
                for nb in range(S // 512):
                    ps = pj.tile([128, 512], F32, tag="pj")
                    for c in range(4):
                        mm(ps, wk_sb[:, c, p * 128:(p + 1) * 128],
                           kT[:, c, nb * 512:(nb + 1) * 512],
                           start=(c == 0), stop=(c == 3))
                    nc.scalar.copy(khT[p][:, nb * 512:(nb + 1) * 512], ps)

            # V -> vh (natural layout, ones column appended per head)
            vT = load_transposed(v_d, S, tag="rawA")  # reuses K slot
            for kc in range(NKC):
                ps = pj.tile([128, 512], F32, tag="pj")
                for c in range(4):
                    mm(ps, vT[:, c, kc * 128:(kc + 1) * 128], wv_sb[:, c, :],
                       start=(c == 0), stop=(c == 3))
                nc.scalar.copy(vh[kc][:, :, 0:DK],
                               ps.rearrange("p (h d) -> p h d", h=H))
                nc.scalar.activation(vh[kc][:, :, DK:DK + 1], vh[kc][:, :, 0:1],
                                     mybir.ActivationFunctionType.Copy,
                                     bias=1.0, scale=0.0)

            # Q -> qhT pairs
            qT = load_transposed(q_d, QH, tag="rawA")
            for p in range(4):
                for nb in range(QH // 512):
                    ps = pj.tile([128, 512], F32, tag="pj")
                    for c in range(4):
                        mm(ps, wq_sb[:, c, p * 128:(p + 1) * 128],
                           qT[:, c, nb * 512:(nb + 1) * 512],
                           start=(c == 0), stop=(c == 3))
                    nc.scalar.copy(qhT[p][:, nb * 512:(nb + 1) * 512], ps)

        # ---------------- phase 3: attention per head ----------------------
        with tc.tile_pool(name="att", bufs=1) as att, \
             tc.tile_pool(name="exps", bufs=NKC) as exps, \
             tc.tile_pool(name="pout", bufs=2) as pout, \
             tc.tile_pool(name="psB", bufs=2, space="PSUM") as psB:

            for h in range(H):
                pr, side = h // 2, h % 2
                po = side * 64
                khT_h = khT[pr][po:po + 64, :]
                qhT_h = qhT[pr][po:po + 64, :]

                av = psB.tile([65, QH], F32, tag="av")
                exp_t = []
                for kc in range(NKC):
                    sc = psB.tile([128, QH], F32, tag="sc")
                    for nb in range(QH // 512):
                        mm(sc[:, nb * 512:(nb + 1) * 512],
                           khT_h[:, kc * 128:(kc + 1) * 128],
                           qhT_h[:, nb * 512:(nb + 1) * 512],
                           start=True, stop=True)
                    et = exps.tile([128, QH], F32R, tag="exp")
                    nc.scalar.activation(et, sc, EXPF, scale=SCALE)
                    # multiplicative {0,1} mask; 12/16 tiles on GpSimd
                    eng = nc.vector if kc % 4 == 0 else nc.gpsimd
                    eng.tensor_tensor(et, et, mt_sb[:, kc, :], MULT)
                    for nb in range(QH // 512):
                        mm(av[:, nb * 512:(nb + 1) * 512], vh[kc][:, h, :],
                           et[:, nb * 512:(nb + 1) * 512],
                           start=(kc == 0), stop=(kc == NKC - 1))
                    exp_t.append(et)

                # denominators -> reciprocal -> broadcast via ones outer-product
                rb = att.tile([128, QH], F32, tag="rb")
                recip = rb[0:1, :]
                nc.vector.tensor_copy(recip, av[64:65, :])
                nc.vector.reciprocal_approx_fast(recip, recip)
                rbp = psB.tile([128, QH], F32, tag="sc")
                for nb in range(QH // 512):
                    nc.tensor.matmul(rbp[:, nb * 512:(nb + 1) * 512], ones_row,
                                     recip[:, nb * 512:(nb + 1) * 512],
                                     start=True, stop=True)
                nc.scalar.copy(rb, rbp)

                # normalized head output (transposed) for the out-projection
                nc.vector.tensor_tensor(headsT[pr][po:po + 64, :],
                                        av[0:64, :], rb[0:64, :], MULT)

                # normalize + store attention probabilities
                for kc in range(NKC):
                    pt = pout.tile([128, QH], F32, tag="pt")
                    nc.vector.tensor_tensor(pt, exp_t[kc], rb, MULT)
                    nc.sync.dma_start(
                        attn_d[h, kc * 128:(kc + 1) * 128, :], pt)

        # ---------------- phase 4: out projection + residual + LayerNorm ---
        with tc.tile_pool(name="ln", bufs=2) as ln, \
             tc.tile_pool(name="psC", bufs=2, space="PSUM") as psC:

            for qt in range(NQT):
                op = psC.tile([128, DM], F32, tag="op")
                for pr in range(4):
                    mm(op, headsT[pr][:, qt * 128:(qt + 1) * 128],
                       wo_sb[:, pr, :], start=(pr == 0), stop=(pr == 3))
                qres = ln.tile([128, DM], F32R, tag="qres")
                nc.scalar.dma_start(qres, q_d[qt * 128:(qt + 1) * 128, :])
                x = ln.tile([128, DM], F32, tag="x")
                nc.vector.tensor_tensor(x, op, qres, ADD)

                ssum = ln.tile([128, 1], F32, tag="ssum")
                nc.vector.reduce_sum(ssum, x, axis=mybir.AxisListType.X)
                nmean = ln.tile([128, 1], F32, tag="nmean")
                nc.vector.tensor_scalar(nmean, ssum, -1.0 / DM, None, MULT)
                xc = ln.tile([128, DM], F32, tag="xc")
                nc.vector.tensor_scalar(xc, x, nmean, None, ADD)
                sq = ln.tile([128, DM], F32, tag="sq")
                ssq = ln.tile([128, 1], F32, tag="ssq")
                nc.scalar.activation(sq, xc, SQUAREF, accum_out=ssq)
                var = ln.tile([128, 1], F32, tag="var")
                nc.vector.tensor_scalar(var, ssq, 1.0 / DM, EPS, MULT, ADD)
                std = ln.tile([128, 1], F32, tag="std")
                nc.scalar.activation(std, var, SQRTF)
                rstd = ln.tile([128, 1], F32, tag="rstd")
                nc.vector.reciprocal(rstd, std)
                xn = ln.tile([128, DM], F32, tag="xn")
                nc.vector.tensor_scalar(xn, xc, rstd, None, MULT)
                xg = ln.tile([128, DM], F32, tag="xg")
                nc.vector.tensor_tensor(xg, xn, gb_sb, MULT)
                xb = ln.tile([128, DM], F32, tag="xb")
                nc.vector.tensor_tensor(xb, xg, bb_sb, ADD)
                nc.sync.dma_start(out_d[qt * 128:(qt + 1) * 128, :], xb)

    nc.compile()
    return nc


def get_program():
    if "nc" not in _prog_cache:
        _prog_cache["nc"] = build_program()
    return _prog_cache["nc"]


def make_in_maps(q, k, v, w_q, w_k, w_v, w_o, ln_g, ln_b, mask):
    """Shard full inputs into the 8 per-core input maps."""
    mt = (~np.asarray(mask, dtype=bool)).astype(np.uint8).transpose(0, 2, 1)
    gb = np.tile(np.asarray(ln_g, np.float32)[None, :], (128, 1))
    bb = np.tile(np.asarray(ln_b, np.float32)[None, :], (128, 1))
    in_maps = []
    for c in range(8):
        b, half = c // 2, c % 2
        q0 = half * QH
        in_maps.append({
            "q": np.ascontiguousarray(q[b, q0:q0 + QH, :], dtype=np.float32),
            "k": np.ascontiguousarray(k[b], dtype=np.float32),
            "v": np.ascontiguousarray(v[b], dtype=np.float32),
            "mt": np.ascontiguousarray(mt[b][:, q0:q0 + QH]),
            "wq": np.asarray(w_q, np.float32), "wk": np.asarray(w_k, np.float32),
            "wv": np.asarray(w_v, np.float32), "wo": np.asarray(w_o, np.float32),
            "gb": gb, "bb": bb, "ident": np.eye(128, dtype=np.float32),
        })
    return in_maps


def assemble(results):
    """Gather per-core outputs into full (out, attn)."""
    out = np.empty((B, S, DM), np.float32)
    attn = np.empty((B, H, S, S), np.float32)
    for c in range(8):
        b, half = c // 2, c % 2
        q0 = half * QH
        out[b, q0:q0 + QH, :] = results[c]["out_part"]
        attn[b, :, q0:q0 + QH, :] = results[c]["attn_part"].transpose(0, 2, 1)
    return out, attn


def kernel(q, k, v, w_q, w_k, w_v, w_o, ln_g, ln_b, mask):
    from concourse.bass_utils import run_bass_kernel_spmd

    nc = get_program()
    in_maps = make_in_maps(q, k, v, w_q, w_k, w_v, w_o, ln_g, ln_b, mask)
    res = run_bass_kernel_spmd(nc, in_maps, list(range(8)))
    return assemble(res.results)
